# revision 1
# baseline (speedup 1.0000x reference)
"""Cross-attention kernel for Trainium2 (8 NeuronCores, data-parallel over batch).

Per core (one batch b):
  q = Wq @ x; k = Wk @ xs; v = Wv @ xs          (channel mix, c=64 contraction)
  per head d:  S^T[g,h] = k_d q_d^T             (contract w)
               P^T = exp(S^T/8 + BIAS)          (no-max softmax; bias keeps fp16 exp in range)
               O[h,w] = P^T.T @ V_d ; Z[h] = P^T.T @ 1 ; out = O / Z

v4 design notes:
- Projection matmuls read a strided bf16 view of the fp32 input tiles
  (high halfwords = truncation): bf16 rate with no cast instructions.
- CH=512: every projection PSUM tile is exactly one bank, so both psum
  tags run 4 buffers deep (8 banks) and the drain latency per tile
  halves - the PE never waits long on psum recycling.
- KV and Q projections are fused; the Q pair packs chunks (it, it+64)
  into PE row groups 0-63/64-127 AND column groups 0-63/64-127.
- PSUM drains are single full-width natural-order [128,512] fp16 copies.
  K/Q staging (Ktc/Qtc) keeps the w-half INTERLEAVED in the middle index
  (g*2+i): every xbar-transpose source is a contiguous 2D block covering
  both w-halves, and one transpose moves a 32-row block (xbars have a
  ~1.3us fixed cost, so they are batched 8 iterations per call).
- Full-width drains also remove the xbar footprint hazard: the transpose
  DMA hardware touches all 128 source partitions regardless of the AP
  (partial-partition writes to a staging tile raced on HW).
- Attention is software-pipelined across heads: S(d+1) issues before
  O(d); vh tiles are persistent (ones column memset once); st/ops PSUM
  tags use 4 bufs each; the two normalize muls split across DVE and ACT.
- Output is written fp16 (halves out DMA) and upcast on host.
"""

import sys

try:
    import concourse  # noqa: F401
except ImportError:  # pragma: no cover
    sys.path.insert(0, "/opt/trn_rl_repo")

import numpy as np

import concourse.bass as bass  # noqa: F401
from concourse import bacc
import concourse.mybir as mybir
import concourse.tile as tile

F32 = mybir.dt.float32
BF16 = mybir.dt.bfloat16
F16 = mybir.dt.float16

B = 8
C = 64
H = 256
W = 256
W2 = W // 2

TEMP_INV = 1.0 / float(np.sqrt(C))
EXP_BIAS = -5.0

CH = 512           # spatial columns per chunk (2 image rows) = 1 PSUM bank
NP = H * W // CH // 2   # 64 fused iterations
XB = 4             # iterations per xbar block (16 K g-rows, 16 Q h-rows)


def build_program(debug_dump=False):
    nc = bacc.Bacc("TRN2", target_bir_lowering=False, debug=False)

    x = nc.dram_tensor("x", [C, H, W], F32, kind="ExternalInput")
    xs = nc.dram_tensor("xs", [C, H, W], F32, kind="ExternalInput")
    wqT = nc.dram_tensor("wqT", [C, C], F32, kind="ExternalInput")
    wkvT = nc.dram_tensor("wkvT", [C, 2 * C], F32, kind="ExternalInput")
    out = nc.dram_tensor("out", [C, H, W], F16, kind="ExternalOutput")
    v_dram = nc.dram_tensor("v_dram", [C, H, W], F16, kind="Internal")

    x_flat = x.rearrange("c h w -> c (h w)")
    xs_flat = xs.rearrange("c h w -> c (h w)")
    v_flat = v_dram.rearrange("c h w -> c (h w)")

    def bf16_hi(ap):
        # strided bf16 view of an fp32 SBUF tile: the high halfword of each
        # fp32 is its bf16 truncation (little endian)
        return ap.bitcast(BF16).rearrange("c (n two) -> c n two", two=2)[:, :, 1]

    with tile.TileContext(nc) as tc:
        with (
            tc.tile_pool(name="consts", bufs=1) as consts,
            tc.tile_pool(name="stage", bufs=1) as stage,
        ):
            # ---- constants (weights duplicated into both partition halves) ----
            wq2f = consts.tile([128, C], F32)
            wkv2f = consts.tile([128, 2 * C], F32)
            for hlf in range(2):
                nc.gpsimd.dma_start(wq2f[hlf * C:(hlf + 1) * C, :], wqT[:])
                nc.gpsimd.dma_start(wkv2f[hlf * C:(hlf + 1) * C, :], wkvT[:])
            wq2 = consts.tile([128, C], BF16)
            wkv2 = consts.tile([128, 2 * C], BF16)
            nc.vector.tensor_copy(out=wq2[:], in_=wq2f[:])
            nc.vector.tensor_copy(out=wkv2[:], in_=wkv2f[:])
            bias_sb = consts.tile([128, 1], F32)
            nc.vector.memset(bias_sb[:], EXP_BIAS)

            # ---- persistent staging (fp16), written by xbar transposes ----
            # middle index is (spatial*2 + w_half): Ktc[w2, g*2+i, d]
            Ktc = stage.tile([W2, 2 * H, C], F16, tag="Ktc", name="Ktc")
            Qtc = stage.tile([W2, 2 * H, C], F16, tag="Qtc", name="Qtc")

            # =================== fused projection phase ===================
            with (
                tc.tile_pool(name="inring", bufs=4) as inring,
                tc.tile_pool(name="comb", bufs=2) as comb,
                tc.tile_pool(name="ps_proj", bufs=4, space="PSUM") as ps_proj,
            ):
                ckv = cqc = None
                for it in range(NP):
                    sub = it % XB
                    blk = it // XB
                    # ---------- KV pair: chunks (2it, 2it+1), one DMA ----------
                    in_kv = inring.tile([C, 2 * CH], F32, tag="inkv", name="in_kv")
                    nc.gpsimd.dma_start(
                        in_kv[:], xs_flat[:, 2 * it * CH:(2 * it + 2) * CH]
                    )
                    in_kv16 = bf16_hi(in_kv)
                    ps_kv = [
                        ps_proj.tile([2 * C, CH], F32, tag="pskv", name="ps_kv",
                                     padded_shape=[128, CH])
                        for _ in range(2)
                    ]
                    for j in range(2):
                        nc.tensor.matmul(
                            ps_kv[j][:],
                            wkv2[0:C, :],
                            in_kv16[:, j * CH:(j + 1) * CH],
                            start=True, stop=True,
                        )
                    # full-width natural drains (K rows 0-63, V rows 64-127)
                    if sub == 0:
                        ckv = comb.tile([128, 2 * XB * CH], F16, tag="ckv", name="ckv")
                    for j in range(2):
                        s2 = 2 * sub + j
                        dst = ckv[:, s2 * CH:(s2 + 1) * CH]
                        if j == 0:
                            nc.scalar.copy(out=dst, in_=ps_kv[j][:])
                        else:
                            nc.vector.tensor_copy(out=dst, in_=ps_kv[j][:])
                    if sub == XB - 1:
                        nc.sync.dma_start(
                            out=v_flat[:, blk * 4096:(blk + 1) * 4096],
                            in_=ckv[C:2 * C, :],
                        )
                        # one xbar: 16 g-rows x 256 w -> Ktc[w2, 32 (g,i), c]
                        nc.scalar.dma_start_transpose(
                            out=Ktc[:, blk * 32:(blk + 1) * 32, :],
                            in_=ckv[0:C, :].rearrange("c (r w) -> c r w", w=W2),
                        )

                    # ---------- Q pair: chunks (2it, 2it+1), one DMA ----------
                    in_q = inring.tile([C, 2 * CH], F32, tag="inq", name="in_q")
                    nc.gpsimd.dma_start(
                        in_q[:], x_flat[:, 2 * it * CH:(2 * it + 2) * CH]
                    )
                    in_q16 = bf16_hi(in_q)
                    ps_q = [
                        ps_proj.tile([C, CH], F32, tag="psq", name="ps_q",
                                     padded_shape=[128, CH])
                        for _ in range(2)
                    ]
                    for j in range(2):
                        nc.tensor.matmul(
                            ps_q[j][:],
                            wq2[0:C, :],
                            in_q16[:, j * CH:(j + 1) * CH],
                            start=True, stop=True,
                        )
                    if sub == 0:
                        cqc = comb.tile([C, 2 * XB * CH], F16, tag="cqc", name="cqc")
                    for j in range(2):
                        s2 = 2 * sub + j
                        dst = cqc[:, s2 * CH:(s2 + 1) * CH]
                        if j == 0:
                            nc.scalar.copy(out=dst, in_=ps_q[j][:])
                        else:
                            nc.vector.tensor_copy(out=dst, in_=ps_q[j][:])
                    if sub == XB - 1:
                        # one xbar: 16 h-rows x 256 w -> Qtc[w2, 32 (h,i), c]
                        nc.sync.dma_start_transpose(
                            out=Qtc[:, blk * 32:(blk + 1) * 32, :],
                            in_=cqc[:, :].rearrange("c (r w) -> c r w", w=W2),
                        )

            if debug_dump:
                ktd = nc.dram_tensor("Ktd", [W2, 2 * H, C], F16, kind="ExternalOutput")
                qtd = nc.dram_tensor("Qtd", [W2, 2 * H, C], F16, kind="ExternalOutput")
                nc.sync.dma_start(out=ktd[:], in_=Ktc[:])
                nc.sync.dma_start(out=qtd[:], in_=Qtc[:])

            # =================== attention ===================
            Ktv = Ktc.rearrange("p (g i) c -> p g i c", i=2)
            Qtv = Qtc.rearrange("p (h i) c -> p h i c", i=2)
            with (
                tc.tile_pool(name="attn", bufs=1) as attn,
                tc.tile_pool(name="ps_attn", bufs=4, space="PSUM") as ps_attn,
            ):
                # persistent vh tiles: [parity][gt], ones column set once
                vh = [
                    [
                        attn.tile([128, W + 1], F16, tag=f"vh{par}{gt}", name="vh")
                        for gt in range(2)
                    ]
                    for par in range(2)
                ]
                for par in range(2):
                    for gt in range(2):
                        nc.gpsimd.memset(vh[par][gt][:, W:W + 1], 1.0)

                def load_v(d):
                    for gt in range(2):
                        nc.gpsimd.dma_start(
                            out=vh[d % 2][gt][:, 0:W],
                            in_=v_dram[d, gt * 128:(gt + 1) * 128, :],
                        )

                def s_exp(d):
                    es = []
                    for gt in range(2):
                        st = ps_attn.tile([128, H], F32, tag="st", name="st")
                        for i in range(2):
                            nc.tensor.matmul(
                                st[:],
                                Ktv[:, gt * 128:(gt + 1) * 128, i, d],
                                Qtv[:, :, i, d],
                                start=(i == 0), stop=(i == 1),
                            )
                        e = attn.tile([128, H], F16, tag="expS", bufs=4, name="expS")
                        nc.scalar.activation(
                            out=e[:], in_=st[:],
                            func=mybir.ActivationFunctionType.Exp,
                            bias=bias_sb[:], scale=TEMP_INV,
                        )
                        es.append(e)
                    return es

                def o_phase(d, es):
                    for hc in range(2):
                        ops = ps_attn.tile([128, W + 1], F32, tag="ops", name="ops")
                        for gt in range(2):
                            nc.tensor.matmul(
                                ops[:],
                                es[gt][:, hc * 128:(hc + 1) * 128],
                                vh[d % 2][gt][:],
                                start=(gt == 0), stop=(gt == 1),
                            )
                        r = attn.tile([128, 1], F32, tag="r", bufs=4, name="r")
                        nc.vector.reciprocal(r[:], ops[:, W:W + 1])
                        osb = attn.tile([128, W], F16, tag="osb", bufs=4, name="osb")
                        if hc == 0:
                            nc.vector.tensor_scalar_mul(osb[:], ops[:, 0:W], r[:])
                        else:
                            nc.scalar.activation(
                                out=osb[:], in_=ops[:, 0:W],
                                func=mybir.ActivationFunctionType.Copy,
                                scale=r[:],
                            )
                        nc.sync.dma_start(
                            out=out[d, hc * 128:(hc + 1) * 128, :], in_=osb[:]
                        )

                # software pipeline: S(d+1) issues before O(d)
                load_v(0)
                load_v(1)
                es_cur = s_exp(0)
                for d in range(C):
                    es_next = s_exp(d + 1) if d + 1 < C else None
                    o_phase(d, es_cur)
                    # prefetch v for d+2: must be issued AFTER o_phase(d),
                    # which is the last reader of the parity-(d%2) vh tiles
                    if d + 2 < C:
                        load_v(d + 2)
                    es_cur = es_next

    nc.compile()
    return nc


_NC_CACHE = None


def _get_program():
    global _NC_CACHE
    if _NC_CACHE is None:
        _NC_CACHE = build_program()
    return _NC_CACHE


def kernel(x, x_s, Wq, Wkv):
    from concourse.bass_utils import run_bass_kernel_spmd

    nc = _get_program()
    wqT = np.ascontiguousarray(Wq.T).astype(np.float32)
    wkvT = np.ascontiguousarray(Wkv.T).astype(np.float32)
    in_maps = [
        {
            "x": np.ascontiguousarray(x[b]),
            "xs": np.ascontiguousarray(x_s[b]),
            "wqT": wqT,
            "wkvT": wkvT,
        }
        for b in range(B)
    ]
    res = run_bass_kernel_spmd(nc, in_maps, list(range(B)))
    return np.stack(
        [res.results[i]["out"].astype(np.float32) for i in range(B)], axis=0
    )



# revision 10
# speedup vs baseline: 2.1064x; 2.1064x over previous
"""Cross-attention kernel for Trainium2 (8 NeuronCores, data-parallel over batch).

Per core (one batch b):
  q = Wq @ x; k = Wk @ xs; v = Wv @ xs          (channel mix, c=64 contraction)
  per head d:  S^T[g,h] = k_d q_d^T             (contract w)
               P^T = exp(S^T/8 + BIAS)          (no-max softmax; bias keeps fp16 exp in range)
               O[h,w] = P^T.T @ V_d ; Z[h] = P^T.T @ 1 ; out = O / Z

v5 design notes (vs v4):
- Inputs arrive bf16 from the host (same numerics as v4's on-chip bf16
  truncation) -> input DMA halves to ~17MB.
- K and Q are computed by TRANSPOSED projection: each 128-spatial input
  chunk [64c, 128s] is the PE stationary operand, the 64x64 weight is the
  moving operand, so the output lands directly as [128 w2, 64 c] chunks in
  the attention staging layout Ktc/Qtc[w2, (g,i), c]. This eliminates the
  v4 xbar transposes entirely (~100us of DMA-pool time).
- Chunk pairs run CONCURRENTLY on PE row groups 0-63/64-127 via
  tile_position (64-contraction matmuls pack 2x). V projection packs its
  two 64-row outputs into one PSUM bank via row+col tile_position.
- Eight K/Q chunk outputs share one PSUM bank (col slices) -> one
  [128,512] drain per bank.
- Attention: S^T for both g-halves accumulates into ONE [128,512] PSUM
  bank -> a single exp activation per head. V is read back in quad-head
  tiles [128, 4, 257] (ones column for Z); output written in quad-head
  [128, 4, 256] tiles to a [H, C, W] dram layout (2KB dma runs), host
  transposes back.
- Software pipeline: S(d+2) issues before O(d); engine split: exp on
  scalar, K-drains on vector, Q-drains on scalar, V-drains + norm on
  gpsimd/vector, input issue on gpsimd, v/out writes on sync.
"""

import sys

try:
    import concourse  # noqa: F401
except ImportError:  # pragma: no cover
    sys.path.insert(0, "/opt/trn_rl_repo")

import numpy as np

import concourse.bass as bass  # noqa: F401
from concourse import bacc
import concourse.mybir as mybir
import concourse.tile as tile

F32 = mybir.dt.float32
BF16 = mybir.dt.bfloat16
F16 = mybir.dt.float16

B = 8
C = 64
H = 256
W = 256
W2 = W // 2
HW = H * W

TEMP_INV = 1.0 / float(np.sqrt(C))
EXP_BIAS = -5.0

IT = 16            # projection outer iterations
SPI = HW // IT     # 4096 spatial positions per iteration


def build_program():
    nc = bacc.Bacc("TRN2", target_bir_lowering=False, debug=False)

    # inputs pre-tiled on host: [it, (j c), s] with j = 2048-col half
    x_b = nc.dram_tensor("x_b", [IT, 128, SPI // 2], BF16, kind="ExternalInput")
    xs_b = nc.dram_tensor("xs_b", [IT, 128, SPI // 2], BF16, kind="ExternalInput")
    wk2d = nc.dram_tensor("wk2d", [128, C], BF16, kind="ExternalInput")
    wq2d = nc.dram_tensor("wq2d", [128, C], BF16, kind="ExternalInput")
    wv2d = nc.dram_tensor("wv2d", [128, C], BF16, kind="ExternalInput")
    out_t = nc.dram_tensor("out_t", [H, C, W], F16, kind="ExternalOutput")
    v_dram = nc.dram_tensor("v_dram", [C, H, W], F16, kind="Internal")

    v_flat = v_dram.rearrange("c h w -> c (h w)")
    v_hcw = v_dram.rearrange("c h w -> h c w")
    x_r = x_b
    xs_r = xs_b

    with tile.TileContext(nc) as tc:
        with (
            tc.tile_pool(name="consts", bufs=1) as consts,
            tc.tile_pool(name="stage", bufs=1) as stage,
        ):
            wk2 = consts.tile([128, C], BF16)
            wq2 = consts.tile([128, C], BF16)
            wv2 = consts.tile([128, C], BF16)
            nc.gpsimd.dma_start(wk2[:], wk2d[:])
            nc.gpsimd.dma_start(wq2[:], wq2d[:])
            nc.gpsimd.dma_start(wv2[:], wv2d[:])
            bias_sb = consts.tile([128, 1], F32)
            nc.vector.memset(bias_sb[:], EXP_BIAS)

            # persistent transposed staging: [w2, (g*2+i), c]
            Ktc = stage.tile([W2, 2 * H, C], F16, tag="Ktc", name="Ktc")
            Qtc = stage.tile([W2, 2 * H, C], F16, tag="Qtc", name="Qtc")

            # =================== projection phase ===================
            with (
                tc.tile_pool(name="inring", bufs=3) as inring,
                tc.tile_pool(name="vstg", bufs=2) as vstg,
                tc.tile_pool(name="ps_proj", bufs=2, space="PSUM") as psp,
            ):
                def tproj(in_t, wmov, dst, it):
                    # transposed projection of one [128, 2048] input tile
                    # into dst[:, it*32:(it+1)*32, :].  The two partition
                    # halves (row groups 0-63 / 64-127) run concurrently.
                    for kb in range(2):
                        psA = psp.tile([128, 512], F32, tag="pst", bufs=4,
                                       name="pst")
                        psB = psp.tile([128, 512], F32, tag="pst", bufs=4,
                                       name="pst")
                        for k in range(8):
                            kk = kb * 8 + k
                            nc.tensor.matmul(
                                psA[:, k * 64:(k + 1) * 64],
                                in_t[0:64, kk * 128:(kk + 1) * 128],
                                wmov[0:64, :],
                                start=True, stop=True, tile_position=(0, 0),
                            )
                            nc.tensor.matmul(
                                psB[:, k * 64:(k + 1) * 64],
                                in_t[64:128, kk * 128:(kk + 1) * 128],
                                wmov[64:128, :],
                                start=True, stop=True, tile_position=(64, 0),
                            )
                        mA = it * 32 + kb * 8
                        mB = it * 32 + 16 + kb * 8
                        if dst is Ktc:
                            nc.vector.tensor_copy(
                                out=dst[:, mA:mA + 8, :], in_=psA[:])
                            nc.vector.tensor_copy(
                                out=dst[:, mB:mB + 8, :], in_=psB[:])
                        else:
                            nc.scalar.copy(out=dst[:, mA:mA + 8, :], in_=psA[:])
                            nc.scalar.copy(out=dst[:, mB:mB + 8, :], in_=psB[:])

                for it in range(IT):
                    in_s = inring.tile([128, 2048], BF16, tag="ins", name="in_s")
                    nc.gpsimd.dma_start(in_s[:], xs_r[it])
                    in_x = inring.tile([128, 2048], BF16, tag="inx", name="in_x")
                    nc.gpsimd.dma_start(in_x[:], x_r[it])

                    # K transposed-projection (drain on vector)
                    tproj(in_s, wk2, Ktc, it)

                    # V normal projection: 4 psum banks, halves packed via
                    # row+col tile_position
                    vstag = vstg.tile([128, 2048], F16, tag="vst", name="vstag")
                    for k4 in range(4):
                        psv = psp.tile([128, 512], F32, tag="psv", name="psv")
                        nc.tensor.matmul(
                            psv[0:64, :], wv2[0:64, :],
                            in_s[0:64, k4 * 512:(k4 + 1) * 512],
                            start=True, stop=True, tile_position=(0, 0),
                        )
                        nc.tensor.matmul(
                            psv[64:128, :], wv2[64:128, :],
                            in_s[64:128, k4 * 512:(k4 + 1) * 512],
                            start=True, stop=True, tile_position=(64, 64),
                        )
                        if k4 % 2 == 0:
                            nc.vector.tensor_copy(
                                out=vstag[:, k4 * 512:(k4 + 1) * 512],
                                in_=psv[:])
                        else:
                            nc.scalar.copy(
                                out=vstag[:, k4 * 512:(k4 + 1) * 512],
                                in_=psv[:])
                    nc.sync.dma_start(
                        out=v_flat[:, it * SPI:it * SPI + 2048],
                        in_=vstag[0:64, :])
                    nc.sync.dma_start(
                        out=v_flat[:, it * SPI + 2048:(it + 1) * SPI],
                        in_=vstag[64:128, :])

                    # Q transposed-projection (drain on scalar)
                    tproj(in_x, wq2, Qtc, it)

            # =================== attention ===================
            Ktv = Ktc.rearrange("p (g i) c -> p g i c", i=2)
            Qtv = Qtc.rearrange("p (h i) c -> p h i c", i=2)
            with (
                tc.tile_pool(name="attn", bufs=1) as attn,
                tc.tile_pool(name="ps_attn", bufs=1, space="PSUM") as psa,
            ):
                # quad-head V tiles: [128 g, 4 heads, 256 w + ones col]
                vq = {}

                def load_v(q):
                    # load heads 4q..4q+4 for both g-halves
                    for gt in range(2):
                        t = attn.tile([128, 4, W + 1], F16, tag=f"vh{gt}",
                                      bufs=3, name="vh")
                        nc.gpsimd.memset(t[:, :, W:W + 1], 1.0)
                        nc.gpsimd.dma_start(
                            out=t[:, :, 0:W],
                            in_=v_hcw[gt * 128:(gt + 1) * 128,
                                      4 * q:4 * q + 4, :],
                        )
                        vq[(q % 3, gt)] = t

                def s_exp(d):
                    st = psa.tile([128, 2 * H], F32, tag="st", bufs=3, name="st")
                    for gt in range(2):
                        for i in range(2):
                            nc.tensor.matmul(
                                st[:, gt * H:(gt + 1) * H],
                                Ktv[:, gt * 128:(gt + 1) * 128, i, d],
                                Qtv[:, :, i, d],
                                start=(i == 0), stop=(i == 1),
                            )
                    e = attn.tile([128, 2 * H], F16, tag="expS", bufs=4,
                                  name="expS")
                    nc.scalar.activation(
                        out=e[:], in_=st[:],
                        func=mybir.ActivationFunctionType.Exp,
                        bias=bias_sb[:], scale=TEMP_INV,
                    )
                    return e

                osb = {}

                def o_phase(d, es):
                    q, j = d // 4, d % 4
                    if j == 0:
                        for h2 in range(2):
                            osb[(h2, q % 2)] = attn.tile(
                                [128, 4, W], F16, tag=f"osb{h2}", bufs=2,
                                name="osb")
                    for hc in range(2):
                        ops = psa.tile([128, W + 1], F32, tag="ops", bufs=4,
                                       name="ops", padded_shape=[128, 512])
                        for gt in range(2):
                            nc.tensor.matmul(
                                ops[:],
                                es[:, gt * H + hc * 128:gt * H + (hc + 1) * 128],
                                vq[(q % 3, gt)][:, j, :],
                                start=(gt == 0), stop=(gt == 1),
                            )
                        r = attn.tile([128, 1], F32, tag="r", bufs=4, name="r")
                        nc.vector.reciprocal(r[:], ops[:, W:W + 1])
                        ob = osb[(hc, q % 2)]
                        if hc == 0:
                            nc.vector.tensor_scalar_mul(
                                ob[:, j, :], ops[:, 0:W], r[:])
                        else:
                            nc.scalar.activation(
                                out=ob[:, j, :], in_=ops[:, 0:W],
                                func=mybir.ActivationFunctionType.Copy,
                                scale=r[:],
                            )
                        if j == 3:
                            nc.sync.dma_start(
                                out=out_t[hc * 128:(hc + 1) * 128,
                                          4 * q:4 * q + 4, :],
                                in_=ob[:],
                            )

                # software pipeline: S(d+2) issues before O(d)
                load_v(0)
                load_v(1)
                load_v(2)
                es_q = [s_exp(0), s_exp(1)]
                for d in range(C):
                    if d + 2 < C:
                        es_q.append(s_exp(d + 2))
                    o_phase(d, es_q.pop(0))
                    # prefetch v quad for heads 4(q+3)..: issued after the
                    # last O reader of the recycled buffer
                    if d % 4 == 3 and (d // 4) + 3 < C // 4:
                        load_v((d // 4) + 3)

    nc.compile()
    return nc


_NC_CACHE = None


def _get_program():
    global _NC_CACHE
    if _NC_CACHE is None:
        _NC_CACHE = build_program()
    return _NC_CACHE


def _make_in_maps(x, x_s, Wq, Wkv):
    import ml_dtypes

    bf = ml_dtypes.bfloat16
    wk = np.ascontiguousarray(Wkv[0:C, :].T)          # [c_in, c_out]
    wv = np.ascontiguousarray(Wkv[C:2 * C, :].T)
    wq = np.ascontiguousarray(Wq.T)
    wk2 = np.vstack([wk, wk]).astype(bf)
    wq2 = np.vstack([wq, wq]).astype(bf)
    wv2 = np.vstack([wv, wv]).astype(bf)
    def tile_in(a):
        # [C, H, W] -> [IT, (j c), 2048]
        return np.ascontiguousarray(
            a.reshape(C, IT, 2, SPI // 2).transpose(1, 2, 0, 3).reshape(
                IT, 128, SPI // 2
            )
        ).astype(bf)

    return [
        {
            "x_b": tile_in(x[b]),
            "xs_b": tile_in(x_s[b]),
            "wk2d": wk2,
            "wq2d": wq2,
            "wv2d": wv2,
        }
        for b in range(B)
    ]


def kernel(x, x_s, Wq, Wkv):
    from concourse.bass_utils import run_bass_kernel_spmd

    nc = _get_program()
    in_maps = _make_in_maps(x, x_s, Wq, Wkv)
    res = run_bass_kernel_spmd(nc, in_maps, list(range(B)))
    return np.stack(
        [
            res.results[i]["out_t"].astype(np.float32).transpose(1, 0, 2)
            for i in range(B)
        ],
        axis=0,
    )
